# revision 1
# baseline (speedup 1.0000x reference)
"""Bass/TRN2 kernel for nn_BMM_S8T_S8N_S8T:
    out[b,m,n] = sat_i8(round(alpha * sum_k a[b,m,k] * b[b,n,k]))
with a: (32, 2048, 64) int8, b: (32, 2048, 64) int8, alpha: f32 scalar.

Sharding: batch dim 32 -> 8 cores x 4 batches (pure data parallel, no
cross-core communication).

Per-core design notes:
  - int8 matmul is not supported by the PE; bf16 x bf16 -> f32 PSUM is exact
    for int8 operands (products < 2^16, sums of 64 < 2^24), so inputs are
    converted to bf16 (and pre-transposed to [K, seq] layout) on host.
  - The 4 local batches are stacked in pairs along SBUF partitions:
    partitions 0-63 hold batch 2p's K=64, partitions 64-127 batch 2p+1's.
    Row-tiled matmuls (tile_position row groups 0 and 2) then run the two
    batches' K=64 contractions concurrently on the 128x128 PE array.
  - Requant drain (PSUM f32 -> SBUF int8, scale + round-half-even + saturate)
    is the bottleneck: only VectorE and ScalarE can read PSUM, at ~1 elem/
    lane/cycle. Both engines drain in parallel on different PSUM banks, in
    [128, 1024] (2-bank) units so fills overlap drains within 8 banks.
    A single tensor_scalar_mul / activation(Copy, scale) instruction does the
    whole requant bit-exactly (verified vs numpy round/clip on HW).
"""

import numpy as np
import ml_dtypes

B, M, N, K = 32, 2048, 2048, 64
NCORES = 8
BPC = B // NCORES          # batches per core (4)
MT = M // 128              # m-tiles per batch (16)
NHALF = 2                  # two 1024-col drain units per m-tile row block
UNIT = N // NHALF          # 1024 columns per drain unit

_CACHE = {}


def _build(alpha: float):
    import concourse.bacc as bacc
    import concourse.mybir as mybir
    from concourse.tile import TileContext

    bf16 = mybir.dt.bfloat16
    f32 = mybir.dt.float32
    i8 = mybir.dt.int8

    nc = bacc.Bacc("TRN2")
    aT = nc.dram_tensor("aT", (BPC // 2, 128, M), bf16, kind="ExternalInput")
    bT = nc.dram_tensor("bT", (BPC // 2, 128, N), bf16, kind="ExternalInput")
    out = nc.dram_tensor("out", (BPC, M, N), i8, kind="ExternalOutput")

    # engine load balancing between the two drain engines (ns per [128,1024]
    # unit, from the TRN2 errata cost model)
    DVE_NS, ACT_NS = 1192.0, 997.0

    with TileContext(nc) as tc:
        with (
            tc.tile_pool(name="inp", bufs=1) as inp_pool,
            tc.tile_pool(name="ps", bufs=4, space="PSUM") as psum_pool,
            tc.tile_pool(name="outp", bufs=6) as out_pool,
        ):
            a_sb = inp_pool.tile([128, BPC // 2, M], bf16, tag="a")
            b_sb = inp_pool.tile([128, BPC // 2, N], bf16, tag="b")
            for p in range(BPC // 2):
                nc.sync.dma_start(out=a_sb[:, p], in_=aT[p])
                nc.sync.dma_start(out=b_sb[:, p], in_=bT[p])

            dve_t = act_t = 0.0
            for p in range(BPC // 2):       # batch pair
                for t in range(MT):         # m tile
                    lhs0 = a_sb[0:64, p, 128 * t : 128 * (t + 1)]
                    lhs1 = a_sb[64:128, p, 128 * t : 128 * (t + 1)]
                    for h in range(NHALF):  # 1024-col drain unit
                        ps0 = psum_pool.tile([128, UNIT], f32, tag="ps")
                        ps1 = psum_pool.tile([128, UNIT], f32, tag="ps")
                        for j in range(2):  # 512-col matmul within unit
                            n0 = UNIT * h + 512 * j
                            c = slice(512 * j, 512 * (j + 1))
                            nc.tensor.matmul(
                                ps0[:, c],
                                lhs0,
                                b_sb[0:64, p, n0 : n0 + 512],
                                start=True,
                                stop=True,
                            )
                            nc.tensor.matmul(
                                ps1[:, c],
                                lhs1,
                                b_sb[64:128, p, n0 : n0 + 512],
                                start=True,
                                stop=True,
                            )
                        for which, ps in ((0, ps0), (1, ps1)):
                            o = out_pool.tile([128, UNIT], i8, tag="o")
                            if dve_t + DVE_NS <= act_t + ACT_NS:
                                nc.vector.tensor_scalar_mul(o[:, :], ps[:, :], alpha)
                                dve_t += DVE_NS
                            else:
                                nc.scalar.activation(
                                    o[:, :],
                                    ps[:, :],
                                    mybir.ActivationFunctionType.Copy,
                                    scale=alpha,
                                )
                                act_t += ACT_NS
                            nc.sync.dma_start(
                                out=out[
                                    2 * p + which,
                                    128 * t : 128 * (t + 1),
                                    UNIT * h : UNIT * (h + 1),
                                ],
                                in_=o[:, :],
                            )
    nc.compile()
    return nc


def kernel(a: np.ndarray, b: np.ndarray, alpha) -> np.ndarray:
    from concourse.bass_utils import run_bass_kernel_spmd

    a = np.asarray(a)
    b = np.asarray(b)
    alpha_f = float(np.asarray(alpha))

    key = alpha_f
    if key not in _CACHE:
        _CACHE[key] = _build(alpha_f)
    nc = _CACHE[key]

    # host-side layout prep: per batch, [seq, K] int8 -> [K, seq] bf16, then
    # stack batch pairs along the partition axis.
    aT = np.ascontiguousarray(a.transpose(0, 2, 1)).astype(ml_dtypes.bfloat16)
    bT = np.ascontiguousarray(b.transpose(0, 2, 1)).astype(ml_dtypes.bfloat16)
    aT = aT.reshape(NCORES, BPC // 2, 128, M)
    bT = bT.reshape(NCORES, BPC // 2, 128, N)

    in_maps = [{"aT": aT[c], "bT": bT[c]} for c in range(NCORES)]
    res = run_bass_kernel_spmd(nc, in_maps, core_ids=list(range(NCORES)))
    outs = [res.results[c]["out"] for c in range(NCORES)]
    return np.concatenate(outs, axis=0).astype(np.int8)


# revision 2
# speedup vs baseline: 1.2038x; 1.2038x over previous
"""Bass/TRN2 kernel for nn_BMM_S8T_S8N_S8T:
    out[b,m,n] = sat_i8(round(alpha * sum_k a[b,m,k] * b[b,n,k]))
with a: (32, 2048, 64) int8, b: (32, 2048, 64) int8, alpha: f32 scalar.

Sharding: batch dim 32 -> 8 cores x 4 batches (pure data parallel, no
cross-core communication).

Per-core design notes:
  - int8 matmul is not supported by the PE; bf16 x bf16 -> f32 PSUM is exact
    for int8 operands (products < 2^16, sums of 64 < 2^24), so inputs are
    converted to bf16 (and pre-transposed to [K, seq] layout) on host.
  - The 4 local batches are stacked in pairs along SBUF partitions:
    partitions 0-63 hold batch 2p's K=64, partitions 64-127 batch 2p+1's.
    Row-tiled matmuls (tile_position row groups 0 and 2) then run the two
    batches' K=64 contractions concurrently on the 128x128 PE array.
  - Requant drain (PSUM f32 -> SBUF int8, scale + round-half-even + saturate)
    is the bottleneck: only VectorE and ScalarE can read PSUM, at ~1 elem/
    lane/cycle. Both engines drain in parallel on different PSUM banks, in
    [128, 1024] (2-bank) units so fills overlap drains within 8 banks.
    A single tensor_scalar_mul / activation(Copy, scale) instruction does the
    whole requant bit-exactly (verified vs numpy round/clip on HW).
"""

import numpy as np
import ml_dtypes

B, M, N, K = 32, 2048, 2048, 64
NCORES = 8
BPC = B // NCORES          # batches per core (4)
MT = M // 128              # m-tiles per batch (16)
NHALF = 2                  # two 1024-col drain units per m-tile row block
UNIT = N // NHALF          # 1024 columns per drain unit

_CACHE = {}


def _build(alpha: float):
    import concourse.bacc as bacc
    import concourse.mybir as mybir
    from concourse.tile import TileContext

    bf16 = mybir.dt.bfloat16
    f32 = mybir.dt.float32
    i8 = mybir.dt.int8

    nc = bacc.Bacc("TRN2")
    aT = nc.dram_tensor("aT", (BPC // 2, 128, M), bf16, kind="ExternalInput")
    bT = nc.dram_tensor("bT", (BPC // 2, 128, N), bf16, kind="ExternalInput")
    out = nc.dram_tensor("out", (BPC, M, N), i8, kind="ExternalOutput")

    # engine load balancing between the two drain engines (ns per [128,1024]
    # unit, hardware-measured)
    DVE_NS, ACT_NS = 1224.0, 1113.0
    WARMUP_MM = 12  # ~5us of dummy matmuls to lift the PE HAM clock gate

    with TileContext(nc) as tc:
        with (
            tc.tile_pool(name="inp", bufs=1) as inp_pool,
            tc.tile_pool(name="ps", bufs=4, space="PSUM") as psum_pool,
            tc.tile_pool(name="outp", bufs=6) as out_pool,
        ):
            # PE warm-up: dense dummy matmuls so the HAM clock-gate lifts the
            # PE to 2.4 GHz before (and while) the input DMAs land.
            wz = inp_pool.tile([128, 512], bf16, tag="wz")
            nc.gpsimd.memset(wz[:, :], 0)
            wps = psum_pool.tile([128, UNIT], f32, tag="ps")
            for _ in range(WARMUP_MM):
                nc.tensor.matmul(
                    wps[:, 0:512], wz[:, 0:128], wz[:, 0:512], start=True, stop=True
                )

            a_sb = inp_pool.tile([128, BPC // 2, M], bf16, tag="a")
            b_sb = inp_pool.tile([128, BPC // 2, N], bf16, tag="b")
            for p in range(BPC // 2):  # pair-0 inputs first so compute starts early
                nc.sync.dma_start(out=a_sb[:, p], in_=aT[p])
                nc.sync.dma_start(out=b_sb[:, p], in_=bT[p])

            dve_t = act_t = 0.0
            for p in range(BPC // 2):       # batch pair
                for t in range(MT):         # m tile
                    lhs0 = a_sb[0:64, p, 128 * t : 128 * (t + 1)]
                    lhs1 = a_sb[64:128, p, 128 * t : 128 * (t + 1)]
                    o0 = out_pool.tile([128, N], i8, tag="o")
                    o1 = out_pool.tile([128, N], i8, tag="o")
                    for h in range(NHALF):  # 1024-col drain unit
                        ps0 = psum_pool.tile([128, UNIT], f32, tag="ps")
                        ps1 = psum_pool.tile([128, UNIT], f32, tag="ps")
                        for j in range(2):  # 512-col matmul within unit
                            n0 = UNIT * h + 512 * j
                            c = slice(512 * j, 512 * (j + 1))
                            nc.tensor.matmul(
                                ps0[:, c],
                                lhs0,
                                b_sb[0:64, p, n0 : n0 + 512],
                                start=True,
                                stop=True,
                            )
                            nc.tensor.matmul(
                                ps1[:, c],
                                lhs1,
                                b_sb[64:128, p, n0 : n0 + 512],
                                start=True,
                                stop=True,
                            )
                        hs = slice(UNIT * h, UNIT * (h + 1))
                        for o, ps in ((o0, ps0), (o1, ps1)):
                            if dve_t + DVE_NS <= act_t + ACT_NS:
                                nc.vector.tensor_scalar_mul(o[:, hs], ps[:, :], alpha)
                                dve_t += DVE_NS
                            else:
                                nc.scalar.activation(
                                    o[:, hs],
                                    ps[:, :],
                                    mybir.ActivationFunctionType.Copy,
                                    scale=alpha,
                                )
                                act_t += ACT_NS
                    for which, o in ((0, o0), (1, o1)):
                        nc.sync.dma_start(
                            out=out[2 * p + which, 128 * t : 128 * (t + 1), :],
                            in_=o[:, :],
                        )
    nc.compile()
    return nc


def kernel(a: np.ndarray, b: np.ndarray, alpha) -> np.ndarray:
    from concourse.bass_utils import run_bass_kernel_spmd

    a = np.asarray(a)
    b = np.asarray(b)
    alpha_f = float(np.asarray(alpha))

    key = alpha_f
    if key not in _CACHE:
        _CACHE[key] = _build(alpha_f)
    nc = _CACHE[key]

    # host-side layout prep: per batch, [seq, K] int8 -> [K, seq] bf16, then
    # stack batch pairs along the partition axis.
    aT = np.ascontiguousarray(a.transpose(0, 2, 1)).astype(ml_dtypes.bfloat16)
    bT = np.ascontiguousarray(b.transpose(0, 2, 1)).astype(ml_dtypes.bfloat16)
    aT = aT.reshape(NCORES, BPC // 2, 128, M)
    bT = bT.reshape(NCORES, BPC // 2, 128, N)

    in_maps = [{"aT": aT[c], "bT": bT[c]} for c in range(NCORES)]
    res = run_bass_kernel_spmd(nc, in_maps, core_ids=list(range(NCORES)))
    outs = [res.results[c]["out"] for c in range(NCORES)]
    return np.concatenate(outs, axis=0).astype(np.int8)


# revision 6
# speedup vs baseline: 1.2352x; 1.0261x over previous
"""Bass/TRN2 kernel for nn_BMM_S8T_S8N_S8T:
    out[b,m,n] = sat_i8(round(alpha * sum_k a[b,m,k] * b[b,n,k]))
with a: (32, 2048, 64) int8, b: (32, 2048, 64) int8, alpha: f32 scalar.

Sharding: batch dim 32 -> 8 cores x 4 batches (pure data parallel, no
cross-core communication).

Per-core design notes:
  - int8 matmul is not supported by the PE; bf16 x bf16 -> f32 PSUM is exact
    for int8 operands (products < 2^16, sums of 64 < 2^24), so inputs are
    converted to bf16 (and pre-transposed to [K, seq] layout) on host.
  - The 4 local batches are stacked in pairs along SBUF partitions:
    partitions 0-63 hold batch 2p's K=64, partitions 64-127 batch 2p+1's.
    Row-tiled matmuls (tile_position row groups 0 and 2) then run the two
    batches' K=64 contractions concurrently on the 128x128 PE array.
  - Requant drain (PSUM f32 -> SBUF int8, scale + round-half-even + saturate)
    is the bottleneck: only VectorE and ScalarE can read PSUM, at ~1 elem/
    lane/cycle. Both engines drain in parallel on different PSUM banks, in
    [128, 1024] (2-bank) units so fills overlap drains within 8 banks.
    A single tensor_scalar_mul / activation(Copy, scale) instruction does the
    whole requant bit-exactly (verified vs numpy round/clip on HW).
"""

import numpy as np
import ml_dtypes

B, M, N, K = 32, 2048, 2048, 64
NCORES = 8
BPC = B // NCORES          # batches per core (4)
MT = M // 128              # m-tiles per batch (16)
NHALF = 2                  # two 1024-col drain units per m-tile row block
UNIT = N // NHALF          # 1024 columns per drain unit

_CACHE = {}


def _build(alpha: float):
    import concourse.bacc as bacc
    import concourse.mybir as mybir
    from concourse.tile import TileContext

    bf16 = mybir.dt.bfloat16
    f32 = mybir.dt.float32
    i8 = mybir.dt.int8

    nc = bacc.Bacc("TRN2")
    aT = nc.dram_tensor("aT", (BPC // 2, 128, M), bf16, kind="ExternalInput")
    bT = nc.dram_tensor("bT", (BPC // 2, 128, N), bf16, kind="ExternalInput")
    out = nc.dram_tensor("out", (BPC, M, N), i8, kind="ExternalOutput")

    # engine load balancing between the two drain engines (ns per [128,1024]
    # unit, hardware-measured)
    DVE_NS, ACT_NS = 1224.0, 1113.0
    WARMUP_MM = 12  # ~5us of dummy matmuls to lift the PE HAM clock gate

    with TileContext(nc) as tc:
        with (
            tc.tile_pool(name="inp", bufs=1) as inp_pool,
            tc.tile_pool(name="ps", bufs=4, space="PSUM") as psum_pool,
            tc.tile_pool(name="outp", bufs=6) as out_pool,
        ):
            # PE warm-up: dense dummy matmuls so the HAM clock-gate lifts the
            # PE to 2.4 GHz before (and while) the input DMAs land. Operand
            # values don't matter (the scratch PSUM bank is never read) but
            # the tile needs a writer; DVE memset boots early.
            wz = inp_pool.tile([128, 512], bf16, tag="wz")
            nc.vector.memset(wz[:, :], 0)
            wps = psum_pool.tile([128, UNIT], f32, tag="ps")
            for _ in range(WARMUP_MM):
                nc.tensor.matmul(
                    wps[:, 0:512], wz[:, 0:128], wz[:, 0:512], start=True, stop=True
                )

            a_sb = inp_pool.tile([128, BPC // 2, M], bf16, tag="a")
            b_sb = inp_pool.tile([128, BPC // 2, N], bf16, tag="b")
            # pair-0 inputs first, issued from engines that boot early, so
            # compute starts as soon as possible
            nc.sync.dma_start(out=a_sb[:, 0], in_=aT[0])
            nc.sync.dma_start(out=b_sb[:, 0], in_=bT[0])
            nc.sync.dma_start(out=a_sb[:, 1], in_=aT[1])
            nc.sync.dma_start(out=b_sb[:, 1], in_=bT[1])

            dve_t = act_t = 0.0
            for p in range(BPC // 2):       # batch pair
                for t in range(MT):         # m tile
                    lhs0 = a_sb[0:64, p, 128 * t : 128 * (t + 1)]
                    lhs1 = a_sb[64:128, p, 128 * t : 128 * (t + 1)]
                    o0 = out_pool.tile([128, N], i8, tag="o")
                    o1 = out_pool.tile([128, N], i8, tag="o")
                    for h in range(NHALF):  # 1024-col drain unit
                        ps0 = psum_pool.tile([128, UNIT], f32, tag="ps")
                        ps1 = psum_pool.tile([128, UNIT], f32, tag="ps")
                        for j in range(2):  # 512-col matmul within unit
                            n0 = UNIT * h + 512 * j
                            c = slice(512 * j, 512 * (j + 1))
                            nc.tensor.matmul(
                                ps0[:, c],
                                lhs0,
                                b_sb[0:64, p, n0 : n0 + 512],
                                start=True,
                                stop=True,
                            )
                            nc.tensor.matmul(
                                ps1[:, c],
                                lhs1,
                                b_sb[64:128, p, n0 : n0 + 512],
                                start=True,
                                stop=True,
                            )
                        hs = slice(UNIT * h, UNIT * (h + 1))
                        for o, ps in ((o0, ps0), (o1, ps1)):
                            if dve_t + DVE_NS <= act_t + ACT_NS:
                                nc.vector.tensor_scalar_mul(o[:, hs], ps[:, :], alpha)
                                dve_t += DVE_NS
                            else:
                                nc.scalar.activation(
                                    o[:, hs],
                                    ps[:, :],
                                    mybir.ActivationFunctionType.Copy,
                                    scale=alpha,
                                )
                                act_t += ACT_NS
                    for which, o in ((0, o0), (1, o1)):
                        nc.sync.dma_start(
                            out=out[2 * p + which, 128 * t : 128 * (t + 1), :],
                            in_=o[:, :],
                        )
    nc.compile()
    return nc


def kernel(a: np.ndarray, b: np.ndarray, alpha) -> np.ndarray:
    from concourse.bass_utils import run_bass_kernel_spmd

    a = np.asarray(a)
    b = np.asarray(b)
    alpha_f = float(np.asarray(alpha))

    key = alpha_f
    if key not in _CACHE:
        _CACHE[key] = _build(alpha_f)
    nc = _CACHE[key]

    # host-side layout prep: per batch, [seq, K] int8 -> [K, seq] bf16, then
    # stack batch pairs along the partition axis.
    aT = np.ascontiguousarray(a.transpose(0, 2, 1)).astype(ml_dtypes.bfloat16)
    bT = np.ascontiguousarray(b.transpose(0, 2, 1)).astype(ml_dtypes.bfloat16)
    aT = aT.reshape(NCORES, BPC // 2, 128, M)
    bT = bT.reshape(NCORES, BPC // 2, 128, N)

    in_maps = [{"aT": aT[c], "bT": bT[c]} for c in range(NCORES)]
    res = run_bass_kernel_spmd(nc, in_maps, core_ids=list(range(NCORES)))
    outs = [res.results[c]["out"] for c in range(NCORES)]
    return np.concatenate(outs, axis=0).astype(np.int8)
